# revision 13
# baseline (speedup 1.0000x reference)
"""Trainium2 Bass kernel for a 2-channel diffusion-reaction PDE step.

Computes, for state = [U; V] on a 4096x4096 grid with constant boundary pads:
    dUdt = a*lap(U) + U - U^3 - V - k
    dVdt = b*lap(V) + U - V
with a = sigmoid(a_org)*0.01, etc., dx = 0.1 (so a*inv_dx2 = sigmoid(a_org)).

Strategy (per NeuronCore, 8 cores, rows sharded 512/core):
  * The 5-point Laplacian + linear cross terms run on the tensor engine as
    bf16 matmuls accumulating in fp32 PSUM:
      - tridiagonal 128x128 weight = vertical (partition-axis) stencil taps,
      - two identity-weight matmuls on column-shifted slices = horizontal
        taps (free-axis shifts are free via AP offsets),
      - a K=2 matmul adds the two halo rows (small side tile),
      - identity-weight matmuls add the -V (resp. +U) cross terms.
    Matmuls are emitted weight-major (same stationary operand for 8 banks in
    a row) so weight reloads amortize/hide.
  * V is only ever consumed in bf16 (its fp32 value never appears in the
    output equations except through the matmuls), so the host supplies V
    pre-cast to bf16 — halving V's HBM traffic and skipping an on-chip cast.
    U is loaded fp32 (needed exactly for the cubic term) and cast to bf16
    on-chip (split across ScalarE/VectorE).
  * The cubic term is fp32: ScalarE Square + fused DVE scalar_tensor_tensor
    t3 = (U^2 - 1) * U = U^3 - U; PSUM evacuation is a second fused stt:
    out_u = (psum + (-k)) - t3.  V evacuates as an ACT copy.
  * Boundary-condition columns/rows are materialized on the host into the
    per-core padded inputs (cols 0 and 4097; halo rows at shard edges).
"""

import numpy as np
import ml_dtypes

import concourse.bass as bass
import concourse.mybir as mybir
from concourse import bacc
from concourse.tile import TileContext
from concourse.bass_utils import run_bass_kernel_spmd
NX, NY = 4096, 4096
NCORES = 8
RPC = NX // NCORES       # 512 rows per core
RT = 128                 # row-tile height (SBUF partitions)
NRT = RPC // RT          # 4 row tiles per core
CT = 512                 # col-tile width (one PSUM bank of fp32)
NCT = NY // CT           # 8 col tiles
W = NY + 2               # padded width (left/right BC columns)

f32 = mybir.dt.float32
bf16 = mybir.dt.bfloat16
ALU = mybir.AluOpType
ACTF = mybir.ActivationFunctionType

# weight tile column layout ([128, 1024] bf16)
W_TRI_U = 0      # cols   0:128  tridiag: off-diag c1, diag -4*c1
W_TRI_V = 128    # cols 128:256  tridiag: off-diag c1v, diag -4*c1v - 1
W_CI_U = 256     # cols 256:384  c1 * I
W_CI_V = 384     # cols 384:512  c1v * I
W_NEG_I = 512    # cols 512:640  -I
W_POS_I = 640    # cols 640:768  +I
W_BND_U = 768    # cols 768:896  rows 0:2, c1 * [e0; e127]
W_BND_V = 896    # cols 896:1024 rows 0:2, c1v * [e0; e127]

_BUILD_CACHE = {}


def _build_nc():
    if "nc" in _BUILD_CACHE:
        return _BUILD_CACHE["nc"]

    nc = bacc.Bacc(None, target_bir_lowering=False)

    u_in = nc.dram_tensor("u_in", [RPC + 2, W], f32, kind="ExternalInput")
    v_in = nc.dram_tensor("v_in", [RPC + 2, W], bf16, kind="ExternalInput")
    wts = nc.dram_tensor("wts", [128, 1024], bf16, kind="ExternalInput")
    kvec = nc.dram_tensor("kvec", [128, 1], f32, kind="ExternalInput")
    out = nc.dram_tensor("out", [2, RPC, NY], f32, kind="ExternalOutput")

    with TileContext(nc) as tc:
        with tc.tile_pool(name="wp", bufs=1) as wp, \
             tc.tile_pool(name="inp", bufs=3) as inp, \
             tc.tile_pool(name="bfp", bufs=3) as bfp, \
             tc.tile_pool(name="sidep", bufs=1) as sidep, \
             tc.tile_pool(name="outp", bufs=2) as outp, \
             tc.tile_pool(name="sqp", bufs=2) as sqp, \
             tc.tile_pool(name="t3p", bufs=9) as t3p, \
             tc.tile_pool(name="psp", bufs=8, space="PSUM") as psp:

            w_t = wp.tile([128, 1024], bf16, tag="w")
            nc.sync.dma_start(out=w_t, in_=wts[:, :])
            kv_t = wp.tile([128, 1], f32, tag="kv")
            nc.sync.dma_start(out=kv_t, in_=kvec[:, :])

            for t in range(NRT):
                r0 = RT * t
                # U rows fp32
                in_t = inp.tile([128, W], f32, tag="in")
                hw_ = W // 2
                nc.sync.dma_start(out=in_t[:, 0:hw_],
                                  in_=u_in[1 + r0:1 + r0 + RT, 0:hw_])
                nc.sync.dma_start(out=in_t[:, hw_:W],
                                  in_=u_in[1 + r0:1 + r0 + RT, hw_:W])
                # bf16 matmul operands: U half cast on-chip, V half DMA'd
                ub_t = bfp.tile([128, 2 * W], bf16, tag="ub")
                for j in range(NCT):
                    ce = min(CT * j + CT + 2, W)
                    nc.vector.tensor_copy(out=ub_t[:, CT * j:ce],
                                          in_=in_t[:, CT * j:ce])
                nc.sync.dma_start(out=ub_t[:, W:2 * W],
                                  in_=v_in[1 + r0:1 + r0 + RT, :])
                # halo rows (prev = r0, next = r0+RT+1); U cast via SWDGE
                side_t = sidep.tile([2, 2 * W], bf16, tag="side")
                nc.gpsimd.dma_start(out=side_t[:, 0:W],
                                    in_=u_in[r0:r0 + RT + 2:RT + 1, :])
                nc.sync.dma_start(out=side_t[:, W:2 * W],
                                  in_=v_in[r0:r0 + RT + 2:RT + 1, :])
                out_t = outp.tile([128, 2 * NY], f32, tag="out")

                # cubic-term prep (independent of matmuls; fills ACT/DVE early)
                t3s = []
                for j in range(NCT):
                    c0 = CT * j
                    uc = in_t[:, c0 + 1:c0 + 1 + CT]
                    u2 = sqp.tile([128, CT], f32, tag="u2")
                    nc.scalar.activation(u2, uc, ACTF.Square)
                    t3 = t3p.tile([128, CT], f32, tag="t3")
                    nc.vector.scalar_tensor_tensor(
                        out=t3, in0=u2, scalar=1.0, in1=uc,
                        op0=ALU.subtract, op1=ALU.mult)
                    t3s.append(t3)

                # ---- U channel: weight-major over 8 PSUM banks ----
                psu = [psp.tile([128, CT], f32, tag="ps", name=f"psu_{t}_{j}")
                       for j in range(NCT)]
                for j in range(NCT):
                    nc.tensor.matmul(psu[j], w_t[:, W_TRI_U:W_TRI_U + 128],
                                     ub_t[:, CT * j + 1:CT * j + 1 + CT],
                                     start=True, stop=False)
                for j in range(NCT):
                    nc.tensor.matmul(psu[j], w_t[:, W_CI_U:W_CI_U + 128],
                                     ub_t[:, CT * j:CT * j + CT],
                                     start=False, stop=False)
                for j in range(NCT):
                    nc.tensor.matmul(psu[j], w_t[:, W_CI_U:W_CI_U + 128],
                                     ub_t[:, CT * j + 2:CT * j + 2 + CT],
                                     start=False, stop=False)
                for j in range(NCT):
                    nc.tensor.matmul(psu[j], w_t[0:2, W_BND_U:W_BND_U + 128],
                                     side_t[0:2, CT * j + 1:CT * j + 1 + CT],
                                     start=False, stop=False)
                for j in range(NCT):
                    nc.tensor.matmul(psu[j], w_t[:, W_NEG_I:W_NEG_I + 128],
                                     ub_t[:, W + CT * j + 1:W + CT * j + 1 + CT],
                                     start=False, stop=True)
                for j in range(NCT):
                    # out_u = (psum + (-k)) - (U^3 - U)
                    nc.vector.scalar_tensor_tensor(
                        out=out_t[:, CT * j:CT * j + CT], in0=psu[j],
                        scalar=kv_t[:, 0:1], in1=t3s[j],
                        op0=ALU.add, op1=ALU.subtract)


                nc.scalar.dma_start(out=out[0, r0:r0 + RT, :],
                                    in_=out_t[:, 0:NY])

                # ---- V channel ----
                psv = [psp.tile([128, CT], f32, tag="ps", name=f"psv_{t}_{j}")
                       for j in range(NCT)]
                for j in range(NCT):
                    nc.tensor.matmul(psv[j], w_t[:, W_TRI_V:W_TRI_V + 128],
                                     ub_t[:, W + CT * j + 1:W + CT * j + 1 + CT],
                                     start=True, stop=False)
                for j in range(NCT):
                    nc.tensor.matmul(psv[j], w_t[:, W_CI_V:W_CI_V + 128],
                                     ub_t[:, W + CT * j:W + CT * j + CT],
                                     start=False, stop=False)
                for j in range(NCT):
                    nc.tensor.matmul(psv[j], w_t[:, W_CI_V:W_CI_V + 128],
                                     ub_t[:, W + CT * j + 2:W + CT * j + 2 + CT],
                                     start=False, stop=False)
                for j in range(NCT):
                    nc.tensor.matmul(psv[j], w_t[0:2, W_BND_V:W_BND_V + 128],
                                     side_t[0:2, W + CT * j + 1:W + CT * j + 1 + CT],
                                     start=False, stop=False)
                for j in range(NCT):
                    nc.tensor.matmul(psv[j], w_t[:, W_POS_I:W_POS_I + 128],
                                     ub_t[:, CT * j + 1:CT * j + 1 + CT],
                                     start=False, stop=True)
                for j in range(NCT):
                    nc.scalar.copy(out_t[:, NY + CT * j:NY + CT * j + CT],
                                   psv[j])

                nc.scalar.dma_start(out=out[1, r0:r0 + RT, :],
                                    in_=out_t[:, NY:2 * NY])

    nc.compile()
    _BUILD_CACHE["nc"] = nc
    return nc


def _sigmoid64(x):
    return 1.0 / (1.0 + np.exp(-np.float64(x)))


def _make_weights(c1, c1v):
    wts = np.zeros((128, 1024), dtype=np.float32)
    idx = np.arange(128)
    tri_u = np.zeros((128, 128), dtype=np.float32)
    tri_u[idx, idx] = -4.0 * c1
    tri_u[idx[:-1], idx[:-1] + 1] = c1
    tri_u[idx[1:], idx[1:] - 1] = c1
    tri_v = np.zeros((128, 128), dtype=np.float32)
    tri_v[idx, idx] = -4.0 * c1v - 1.0
    tri_v[idx[:-1], idx[:-1] + 1] = c1v
    tri_v[idx[1:], idx[1:] - 1] = c1v
    wts[:, W_TRI_U:W_TRI_U + 128] = tri_u
    wts[:, W_TRI_V:W_TRI_V + 128] = tri_v
    wts[idx, W_CI_U + idx] = c1
    wts[idx, W_CI_V + idx] = c1v
    wts[idx, W_NEG_I + idx] = -1.0
    wts[idx, W_POS_I + idx] = 1.0
    wts[0, W_BND_U + 0] = c1
    wts[1, W_BND_U + 127] = c1
    wts[0, W_BND_V + 0] = c1v
    wts[1, W_BND_V + 127] = c1v
    return wts.astype(ml_dtypes.bfloat16)


def _make_in_maps(state, bc, a_org, b_org, k_org):
    c1 = np.float32(_sigmoid64(a_org))       # a * inv_dx2 == sigmoid(a_org)
    c1v = np.float32(_sigmoid64(b_org))
    k = np.float32(_sigmoid64(k_org) * 0.01)

    wts = _make_weights(c1, c1v)
    kvec = np.full((128, 1), -k, dtype=np.float32)

    st = np.asarray(state, dtype=np.float32)[0]        # [2, NX, NY]
    bc = np.asarray(bc, dtype=np.float32)

    in_maps = []
    for c in range(NCORES):
        r0 = RPC * c
        uvc = np.empty((2, RPC + 2, W), dtype=np.float32)
        uvc[:, 1:RPC + 1, 1:NY + 1] = st[:, r0:r0 + RPC, :]
        # halo rows
        if c == 0:
            uvc[0, 0, 1:NY + 1] = bc[0, 0, 2]          # top BC for U
            uvc[1, 0, 1:NY + 1] = bc[0, 1, 2]
        else:
            uvc[:, 0, 1:NY + 1] = st[:, r0 - 1, :]
        if c == NCORES - 1:
            uvc[0, RPC + 1, 1:NY + 1] = bc[0, 0, 3]    # bottom BC for U
            uvc[1, RPC + 1, 1:NY + 1] = bc[0, 1, 3]
        else:
            uvc[:, RPC + 1, 1:NY + 1] = st[:, r0 + RPC, :]
        # left/right BC columns
        uvc[0, :, 0] = bc[0, 0, 0]
        uvc[0, :, NY + 1] = bc[0, 0, 1]
        uvc[1, :, 0] = bc[0, 1, 0]
        uvc[1, :, NY + 1] = bc[0, 1, 1]
        in_maps.append({
            "u_in": uvc[0],
            "v_in": uvc[1].astype(ml_dtypes.bfloat16),
            "wts": wts,
            "kvec": kvec,
        })
    return in_maps


def _run(in_maps, trace=False, **kwargs):
    nc = _build_nc()
    return run_bass_kernel_spmd(nc, in_maps, list(range(NCORES)),
                                trace=trace, **kwargs)


def kernel(state, bc, a_org, b_org, k_org):
    in_maps = _make_in_maps(state, bc, a_org, b_org, k_org)
    res = _run(in_maps).results
    full = np.empty((1, 2, NX, NY), dtype=np.float32)
    for c in range(NCORES):
        full[0, :, RPC * c:RPC * (c + 1), :] = res[c]["out"]
    return full
